# revision 39
# baseline (speedup 1.0000x reference)
"""Trainium2 Bass kernel for CentroidsFlowAD (retrieval_knn, K=1).

Math: for each embedding row e (B*N rows of dim D=1024) and centroid bank
C [M=2048, D], the reference computes min_m sqrt(max(||e||^2 + ||c_m||^2
- 2 e.c_m, 0)). With K_NEIGHBORS=1 the softmin weighting is exactly 1, so
the output is just the distance to the nearest centroid, reshaped to
[B, 1, 56, 56].

Strategy (data-parallel over batch across 8 cores, centroids replicated):
  - host: split embeds by batch (4 samples -> 12544 rows per core), cast
    to fp8e4 and transpose to [D, R]; precompute ||e||^2 (fp32). The
    centroids are SORTED by ||c||^2 (the min over m is permutation
    invariant) with the 128 extreme-||c||^2 centroids moved to the last
    columns; the 1920 mid columns form 240 aligned groups of 8 whose
    ||c||^2/2 agree to ~+-1 (midrange bias b_g per group), the tail gets
    exact per-centroid biases.
  - device (mode="gsort"): per 128-row tile, PSUM[128r, 2048c] = e.c via
    16 fp8 DoubleRow matmuls (2 k-chunks per instruction, ~157 TF/s).
    DVE: one segmented reduce [128, 240g, 8] -> per-group maxes of raw
    cross (single 1 elem/cyc PSUM pass, no per-tile subtraction), plus an
    exact 128-wide sub+reduce for the extreme tail. Per 14-tile block:
    hmax = max(max_g(gm - b_g), hx) ~= max_m(e.c - ||c||^2/2), biased by
    CENTER to keep the bf16 b_g exact-ish. Epilogue:
    sqrt(max(feat + 2*CENTER - 2*hmax, eps)), Newton-refined ACT sqrt.
  - device (mode="pe_bias"): instead accumulates -csq/2 into PSUM via a
    K=1 DoubleRow pair (64*q0 + 8*q1 fp8 decomposition) at +25% PE cost;
    DVE does one full reduce from PSUM. Kept for A/B testing.
  - host: gather per-core [128, NT] outputs, reshape (no unpermute needed
    - only distance values are returned, invariant to centroid order).

Engine budget per core (98 tiles, HW-measured primitives): PE 16 DoubleRow
matmuls/tile -> ~321us; DVE (1920+128+128)c/tile + block finalize
-> ~273us; ACT/DMA hidden (et is fully SBUF-resident); the 3-op block
finalize is software-pipelined into the next block's tiles so it never
stalls the PSUM-releasing reduces. Measured ~380-400us/iter on the 8-core
SPMD run by the unbiased two-K slope method (the bf16 baseline re-measures
~1.11ms by the same method). Max rel err 3.0e-3 vs the fp32
reference (tolerance 2e-2): fp8 cross noise ~2.3e-3, group-bias ~1e-3.
"""

import numpy as np
import ml_dtypes

import concourse.bass as bass
import concourse.mybir as mybir
import concourse.tile as tile
from concourse import bacc
from concourse.bass_utils import run_bass_kernel_spmd

# Problem constants (hardcoded per harness contract)
B, N, D, M = 32, 3136, 1024, 2048
N_CORES = 8
B_PER_CORE = B // N_CORES            # 4
R = B_PER_CORE * N                   # 12544 rows per core
NT = R // 128                        # 98 row tiles per core
KC = D // 128                        # 8 contraction chunks
KP = KC // 2                         # 4 DoubleRow k-pair chunks
NC_CHUNKS = M // 512                 # 4 PSUM chunks of 512 centroids
FP_H = 56
XW = 128                             # extreme-csq centroids handled exactly
GSZ = 8                              # centroids per group (gsort mode)
MMID = M - XW                        # grouped centroids
G = MMID // GSZ                      # 240 groups

BF16 = mybir.dt.bfloat16
FP8 = mybir.dt.float8e4
F32 = mybir.dt.float32
NP_BF16 = ml_dtypes.bfloat16
NP_FP8 = ml_dtypes.float8_e4m3

BIAS_S0 = 64.0   # coarse bias term scale (pe_bias mode)
BIAS_S1 = 8.0    # residual bias term scale (pe_bias mode)
CENTER = 512.0   # group-bias centering (gsort mode, keeps brep bf16 exact-ish)

MODE = "gsort"   # "gsort" | "gsact" | "pe_bias"


def build_program(n_row_tiles=NT, block_tiles=7, n_iters=1, n_devices=N_CORES,
                  enable_asserts=False, mode=None):
    """Build + compile the SPMD bass program.

    n_row_tiles: row tiles (128 rows each) processed per core.
    block_tiles: row tiles per DMA block (must divide n_row_tiles).
    n_iters: repeat whole compute (for loop-delta timing), python-unrolled.
    """
    mode = mode or MODE
    assert n_row_tiles % block_tiles == 0
    n_blocks = n_row_tiles // block_tiles
    rows = n_row_tiles * 128
    blk = block_tiles * 128

    nc = bacc.Bacc("TRN2", target_bir_lowering=False, debug=False,
                   num_devices=n_devices, enable_asserts=enable_asserts)

    et = nc.dram_tensor("et", [D, rows], FP8, kind="ExternalInput").ap()
    ct = nc.dram_tensor("ct", [D, M], FP8, kind="ExternalInput").ap()
    bias = nc.dram_tensor("bias", [1, 2, M], FP8, kind="ExternalInput").ap()
    bones = nc.dram_tensor("bones", [1, 2, 128], FP8, kind="ExternalInput").ap()
    brep = nc.dram_tensor("brep", [128, block_tiles, G], BF16,
                          kind="ExternalInput").ap()
    csqx = nc.dram_tensor("csqx", [128, XW], F32, kind="ExternalInput").ap()
    biasx = nc.dram_tensor("biasx", [1, 2, 512], FP8,
                           kind="ExternalInput").ap()
    feat = nc.dram_tensor("feat", [128, n_row_tiles], F32,
                          kind="ExternalInput").ap()
    out = nc.dram_tensor("out", [128, n_row_tiles], F32,
                         kind="ExternalOutput").ap()

    with tile.TileContext(nc) as tc:
        psum_bufs = 4 if mode == "gsort" else 2
        with (
            tc.tile_pool(name="const", bufs=1) as const_pool,
            tc.tile_pool(name="etp", bufs=2) as et_pool,
            tc.tile_pool(name="psum", bufs=psum_bufs, space="PSUM") as psum_pool,
            tc.tile_pool(name="gmp", bufs=2) as gm_pool,
            tc.tile_pool(name="hm", bufs=2) as hm_pool,
            tc.tile_pool(name="epi", bufs=1) as epi_pool,
        ):
            ct_sb = const_pool.tile([128, KC, M], FP8)
            bias_sb = const_pool.tile([1, 2, M], FP8)
            bones_sb = const_pool.tile([1, 2, 128], FP8)
            brep_sb = const_pool.tile([128, block_tiles, G], BF16)
            csqx_sb = const_pool.tile([128, XW], F32)
            feat_sb = const_pool.tile([128, n_row_tiles], F32)
            hmax_sb = const_pool.tile([128, n_row_tiles], F32)
            # et fully resident (12.8 MB < SBUF): block-granular DMAs so the
            # first tiles only wait for their own block, not the whole load
            eta_sb = const_pool.tile([128, KC, rows], FP8)
            for b in range(n_blocks):
                for k in range(KC):
                    nc.sync.dma_start(
                        eta_sb[:, k, b * blk:(b + 1) * blk],
                        et[k * 128:(k + 1) * 128, b * blk:(b + 1) * blk])
            for k in range(KC):
                nc.sync.dma_start(ct_sb[:, k, :], ct[k * 128:(k + 1) * 128, :])
            if mode == "pe_bias":
                nc.sync.dma_start(bias_sb[:], bias[:, :, :])
                nc.sync.dma_start(bones_sb[:], bones[:, :, :])
            else:
                nc.sync.dma_start(brep_sb[:], brep[:, :, :])
                nc.sync.dma_start(csqx_sb[:], csqx[:, :])
            nc.sync.dma_start(feat_sb[:], feat[:, :])

            def _fin_ops(b, gm, hx):
                """3-step finalize for block b: yields one DVE op per call,
                interleaved between the NEXT block's tile reduces so the
                burst never queues ahead of PSUM-releasing ops."""
                s, w = b * block_tiles, block_tiles
                hm = hm_pool.tile([128, w, G], F32)
                nc.vector.tensor_sub(hm[:], gm[:], brep_sb[:])
                yield
                hg = hm_pool.tile([128, w], F32)
                nc.vector.tensor_reduce(
                    hg[:], hm[:],
                    mybir.AxisListType.X, mybir.AluOpType.max)
                yield
                nc.vector.tensor_tensor(
                    hmax_sb[:, s:s + w], hg[:], hx[:],
                    mybir.AluOpType.max)

            def body(_it=None):
                pend = [None]

                def _step():
                    if pend[0] is not None:
                        try:
                            next(pend[0])
                        except StopIteration:
                            pend[0] = None

                def _flush():
                    while pend[0] is not None:
                        _step()

                for b in range(n_blocks):
                    if mode != "gsort":
                        et_sb = et_pool.tile([128, KC, blk], FP8)
                        for k in range(KC):
                            nc.sync.dma_start(
                                et_sb[:, k, :],
                                et[k * 128:(k + 1) * 128,
                                   b * blk:(b + 1) * blk])
                    gm = gm_pool.tile([128, block_tiles, G], BF16)
                    hx = gm_pool.tile([128, block_tiles], F32)
                    for j in range(block_tiles):
                        t = b * block_tiles + j
                        if mode == "gsort":
                            # half-width PSUM tiles (4 bufs): PE fills one
                            # half while DVE drains the other — this is what
                            # lets the DVE PSUM pass hide under the PE
                            lhsTs = [eta_sb[:, 2 * kp:2 * kp + 2,
                                            t * 128:(t + 1) * 128]
                                     for kp in range(KP)]
                            for h in range(2):
                                ph = psum_pool.tile([128, M // 2], F32)
                                for kp in range(KP):
                                    for n in range(2):
                                        nc.tensor.matmul(
                                            ph[:, n * 512:(n + 1) * 512],
                                            lhsTs[kp],
                                            ct_sb[:, 2 * kp:2 * kp + 2,
                                                  (2 * h + n) * 512:
                                                  (2 * h + n + 1) * 512],
                                            start=(kp == 0),
                                            stop=(kp == KP - 1),
                                            perf_mode=(
                                                mybir.MatmulPerfMode.DoubleRow))
                                if h == 0:
                                    nc.vector.tensor_reduce(
                                        gm[:, j, 0:128],
                                        ph[:].rearrange(
                                            "p (g i) -> p g i", g=128),
                                        mybir.AxisListType.X,
                                        mybir.AluOpType.max)
                                else:
                                    nc.vector.tensor_reduce(
                                        gm[:, j, 128:G],
                                        ph[:, 0:MMID - 1024].rearrange(
                                            "p (g i) -> p g i", g=G - 128),
                                        mybir.AxisListType.X,
                                        mybir.AluOpType.max)
                                    hxb = hm_pool.tile([128, XW], F32)
                                    nc.vector.tensor_sub(
                                        hxb[:], ph[:, MMID - 1024:],
                                        csqx_sb[:])
                                    nc.vector.tensor_reduce(
                                        hx[:, j:j + 1], hxb[:],
                                        mybir.AxisListType.X,
                                        mybir.AluOpType.max)
                            _step()
                            continue
                        ps = psum_pool.tile([128, M], F32)
                        if mode == "pe_bias":
                            # PSUM := -csq/2 (one K=1 DoubleRow pair/chunk)
                            for n in range(NC_CHUNKS):
                                nc.tensor.matmul(
                                    ps[:, n * 512:(n + 1) * 512],
                                    bones_sb[:, :, :],
                                    bias_sb[:, :, n * 512:(n + 1) * 512],
                                    start=True, stop=False,
                                    perf_mode=mybir.MatmulPerfMode.DoubleRow)
                        # PSUM += e.c (4 DoubleRow k-pair chunks)
                        for kp in range(KP):
                            lhsT = et_sb[:, 2 * kp:2 * kp + 2,
                                         j * 128:(j + 1) * 128]
                            for n in range(NC_CHUNKS):
                                nc.tensor.matmul(
                                    ps[:, n * 512:(n + 1) * 512],
                                    lhsT,
                                    ct_sb[:, 2 * kp:2 * kp + 2,
                                          n * 512:(n + 1) * 512],
                                    start=(mode != "pe_bias" and kp == 0),
                                    stop=(kp == KP - 1),
                                    perf_mode=mybir.MatmulPerfMode.DoubleRow)
                        if mode == "pe_bias":
                            nc.vector.tensor_reduce(
                                hmax_sb[:, t:t + 1], ps[:],
                                mybir.AxisListType.X, mybir.AluOpType.max)
                        else:
                            src = ps
                            if mode == "gsact":
                                # ACT evacuates PSUM; DVE works from SBUF
                                hb = hm_pool.tile([128, M], BF16)
                                nc.scalar.copy(hb[:, 0:1024], ps[:, 0:1024])
                                nc.scalar.copy(hb[:, 1024:M], ps[:, 1024:M])
                                src = hb
                            # per-group maxes of raw cross, single pass
                            nc.vector.tensor_reduce(
                                gm[:, j, :],
                                src[:, 0:MMID].rearrange(
                                    "p (g i) -> p g i", g=G),
                                mybir.AxisListType.X, mybir.AluOpType.max)
                            # exact pass over the extreme-csq tail columns
                            hxb = hm_pool.tile([128, XW], F32)
                            nc.vector.tensor_sub(
                                hxb[:], src[:, MMID:M], csqx_sb[:])
                            nc.vector.tensor_reduce(
                                hx[:, j:j + 1], hxb[:],
                                mybir.AxisListType.X, mybir.AluOpType.max)
                    if mode == "gsort":
                        _flush()
                        pend[:] = [_fin_ops(b, gm, hx)]
                    elif mode == "gsact":
                        # finalize block: hmax = max(max_g(gm - b_g), hx)
                        s, w = b * block_tiles, block_tiles
                        hm = hm_pool.tile([128, w, G], F32)
                        nc.vector.tensor_sub(hm[:], gm[:], brep_sb[:])
                        hg = hm_pool.tile([128, w], F32)
                        nc.vector.tensor_reduce(
                            hg[:], hm[:],
                            mybir.AxisListType.X, mybir.AluOpType.max)
                        nc.vector.tensor_tensor(
                            hmax_sb[:, s:s + w], hg[:], hx[:],
                            mybir.AluOpType.max)

                _flush()

                # epilogue: dist = sqrt(max(feat - 2*hmax, eps)), Newton-refined
                d2 = epi_pool.tile([128, n_row_tiles], F32)
                nc.vector.scalar_tensor_tensor(
                    out=d2[:], in0=hmax_sb[:], scalar=-2.0, in1=feat_sb[:],
                    op0=mybir.AluOpType.mult, op1=mybir.AluOpType.add)
                d2c = epi_pool.tile([128, n_row_tiles], F32)
                nc.vector.tensor_scalar_max(d2c[:], d2[:], 1.0e-12)
                s0 = epi_pool.tile([128, n_row_tiles], F32)
                nc.scalar.activation(s0[:], d2c[:],
                                     mybir.ActivationFunctionType.Sqrt)
                rcp = epi_pool.tile([128, n_row_tiles], F32)
                nc.vector.reciprocal(rcp[:], s0[:])
                q = epi_pool.tile([128, n_row_tiles], F32)
                nc.vector.tensor_mul(q[:], d2c[:], rcp[:])
                sq = epi_pool.tile([128, n_row_tiles], F32)
                nc.vector.tensor_add(sq[:], s0[:], q[:])
                res = epi_pool.tile([128, n_row_tiles], F32)
                nc.vector.tensor_scalar_mul(res[:], sq[:], 0.5)
                nc.sync.dma_start(out[:, :], res[:])

            # python-unrolled repetitions (For_i's back-edge machinery has
            # crashed the exec unit on this terminal; unrolled is safe)
            for _ in range(n_iters):
                body()

    nc.compile()
    return nc


_NC_CACHE = {}


def _get_program(key=(NT, 7, 1, N_CORES)):
    if key not in _NC_CACHE:
        _NC_CACHE[key] = build_program(*key)
    return _NC_CACHE[key]


def _bias_rows(csq):
    """Two-term fp8 decomposition of -||c||^2/2: BIAS_S0*q0 + BIAS_S1*q1."""
    x = (-0.5 * csq).astype(np.float32)
    q0 = (x / BIAS_S0).astype(NP_FP8)
    r = x - BIAS_S0 * q0.astype(np.float32)
    q1 = (r / BIAS_S1).astype(NP_FP8)
    return np.stack([q0, q1])[None]                                # [1, 2, M]


def prep_const(centroids, block_tiles=7):
    """Centroid-side tensors: sort by csq (extremes last), fp8 transpose,
    per-group midrange biases + exact biases for the extreme tail."""
    c64 = np.asarray(centroids).astype(np.float64)
    csq0 = np.einsum("md,md->m", c64, c64)
    order = np.argsort(csq0, kind="stable")
    order = np.concatenate(
        [order[XW // 2:-XW // 2], order[:XW // 2], order[-XW // 2:]])
    c64 = c64[order]
    csq = csq0[order]
    ct_np = np.ascontiguousarray(c64.astype(np.float32).astype(NP_FP8).T)
    bias_np = _bias_rows(csq)
    bones_np = np.empty((1, 2, 128), NP_FP8)
    bones_np[0, 0, :] = NP_FP8(BIAS_S0)
    bones_np[0, 1, :] = NP_FP8(BIAS_S1)
    half = 0.5 * csq[:MMID].reshape(MMID // GSZ, GSZ)
    b_g = 0.5 * (half.min(1) + half.max(1)) - CENTER               # midrange
    brep_np = np.ascontiguousarray(np.broadcast_to(
        b_g.astype(np.float32)[None, None, :],
        (128, block_tiles, G))).astype(NP_BF16)
    csqx_np = np.ascontiguousarray(np.broadcast_to(
        (0.5 * csq[MMID:] - CENTER).astype(np.float32)[None, :], (128, XW)))
    biasx_np = np.zeros((1, 2, 512), NP_FP8)
    biasx_np[:, :, 512 - XW:] = _bias_rows(csq[MMID:])
    return {"ct": ct_np, "bias": bias_np, "bones": bones_np,
            "brep": brep_np, "csqx": csqx_np, "biasx": biasx_np}


def prep_rows(e_rows, n_row_tiles=NT):
    """Embedding-side tensors for one core: fp8 transpose + ||e||^2."""
    e = np.asarray(e_rows)
    et_np = np.ascontiguousarray(e.astype(NP_FP8).T)               # [D, R]
    f = np.einsum("rd,rd->r", e.astype(np.float64),
                  e.astype(np.float64)).astype(np.float32)
    if MODE in ("gsort", "gsact"):
        f = f + 2.0 * CENTER     # hmax comes back CENTER-shifted
    feat_np = np.ascontiguousarray(f.reshape(n_row_tiles, 128).T)
    return {"et": et_np, "feat": feat_np}


def prep_inputs(embeds, centroids):
    """Host-side shard + layout prep. Returns per-core input maps."""
    embeds = np.asarray(embeds)
    const = prep_const(centroids)
    in_maps = []
    for c in range(N_CORES):
        e = embeds[c * B_PER_CORE:(c + 1) * B_PER_CORE].reshape(R, D)
        m = dict(const)
        m.update(prep_rows(e))
        in_maps.append(m)
    return in_maps


def gather_output(results):
    """results: list of 8 dicts with 'out' [128, NT] -> [B, 1, 56, 56]."""
    per_core = [np.asarray(r["out"]).T.reshape(R) for r in results]
    sim = np.concatenate(per_core).reshape(B, N)
    return sim.reshape(B, FP_H, FP_H)[:, None, :, :].astype(np.float32)


def kernel(embeds, centroids):
    nc = _get_program()
    in_maps = prep_inputs(embeds, centroids)
    res = run_bass_kernel_spmd(nc, in_maps, list(range(N_CORES)))
    return gather_output(res.results)


class CachedRunner:
    """Low-overhead repeat runner: jit once, keep inputs resident on device.

    Mirrors bass2jax.run_bass_via_pjrt's multi-core path but caches the
    jitted callable and the device-side input shards so repeated calls pay
    only dispatch + execution (for timing measurements).
    """

    def __init__(self, nc, in_maps):
        import jax
        import concourse.mybir as _mybir
        from jax.sharding import Mesh, PartitionSpec, NamedSharding
        from jax.experimental.shard_map import shard_map
        from concourse import bass2jax

        bass2jax.install_neuronx_cc_hook()
        n_cores = len(in_maps)
        partition_name = (nc.partition_id_tensor.name
                          if nc.partition_id_tensor else None)
        in_names, out_names, out_avals = [], [], []
        for alloc in nc.m.functions[0].allocations:
            if not isinstance(alloc, _mybir.MemoryLocationSet):
                continue
            name = alloc.memorylocations[0].name
            if alloc.kind == "ExternalInput":
                if name != partition_name:
                    in_names.append(name)
            elif alloc.kind == "ExternalOutput":
                shape = tuple(alloc.tensor_shape)
                dtype = _mybir.dt.np(alloc.dtype)
                out_names.append(name)
                out_avals.append(jax.core.ShapedArray(shape, dtype))
        n_params = len(in_names)
        all_in = in_names + out_names
        if partition_name is not None:
            all_in.append(partition_name)

        def _body(*args):
            operands = list(args)
            if partition_name is not None:
                operands.append(bass2jax.partition_id_tensor())
            outs = bass2jax._bass_exec_p.bind(
                *operands,
                out_avals=tuple(out_avals),
                in_names=tuple(all_in),
                out_names=tuple(out_names),
                lowering_input_output_aliases=(),
                sim_require_finite=True,
                sim_require_nnan=True,
                nc=nc,
            )
            return tuple(outs)

        devices = jax.devices()[:n_cores]
        mesh = Mesh(np.asarray(devices), ("core",))
        n_outs = len(out_names)
        donate = tuple(range(n_params, n_params + n_outs))
        self._fn = jax.jit(
            shard_map(_body, mesh=mesh,
                      in_specs=(PartitionSpec("core"),) * (n_params + n_outs),
                      out_specs=(PartitionSpec("core"),) * n_outs,
                      check_rep=False),
            donate_argnums=donate, keep_unused=True)
        sh = NamedSharding(mesh, PartitionSpec("core"))
        self._dev_in = [
            jax.device_put(
                np.concatenate([np.asarray(in_maps[c][nm])
                                for c in range(n_cores)], axis=0), sh)
            for nm in in_names]
        self._zero_shapes = [(n_cores * a.shape[0], *a.shape[1:])
                             for a in out_avals]
        self._zero_dtypes = [a.dtype for a in out_avals]
        self._out_names = out_names
        self._out_avals = out_avals
        self._n_cores = n_cores
        self._jax = jax

    def __call__(self):
        zeros = [np.zeros(s, d) for s, d in
                 zip(self._zero_shapes, self._zero_dtypes)]
        out = self._fn(*self._dev_in, *zeros)
        self._jax.block_until_ready(out)
        return out

    def results(self):
        out = self()
        return [
            {nm: np.asarray(out[i]).reshape(
                self._n_cores, *self._out_avals[i].shape)[c]
             for i, nm in enumerate(self._out_names)}
            for c in range(self._n_cores)]


# revision 40
# speedup vs baseline: 1.1136x; 1.1136x over previous
"""Trainium2 Bass kernel for CentroidsFlowAD (retrieval_knn, K=1).

Math: for each embedding row e (B*N rows of dim D=1024) and centroid bank
C [M=2048, D], the reference computes min_m sqrt(max(||e||^2 + ||c_m||^2
- 2 e.c_m, 0)). With K_NEIGHBORS=1 the softmin weighting is exactly 1, so
the output is just the distance to the nearest centroid, reshaped to
[B, 1, 56, 56].

Strategy (data-parallel over batch across 8 cores, centroids replicated):
  - host: split embeds by batch (4 samples -> 12544 rows per core), cast
    to fp8e4 and transpose to [D, R]; precompute ||e||^2 (fp32). The
    centroids are SORTED by ||c||^2 (the min over m is permutation
    invariant) with the 128 extreme-||c||^2 centroids moved to the last
    columns; the 1920 mid columns form 240 aligned groups of 8 whose
    ||c||^2/2 agree to ~+-1 (midrange bias b_g per group), the tail gets
    exact per-centroid biases.
  - device (mode="gsort"): per 128-row tile, PSUM[128r, 2048c] = e.c via
    16 fp8 DoubleRow matmuls (2 k-chunks per instruction, ~157 TF/s).
    DVE: one segmented reduce [128, 240g, 8] -> per-group maxes of raw
    cross (single 1 elem/cyc PSUM pass, no per-tile subtraction), plus an
    exact 128-wide sub+reduce for the extreme tail. Per 14-tile block:
    hmax = max(max_g(gm - b_g), hx) ~= max_m(e.c - ||c||^2/2), biased by
    CENTER to keep the bf16 b_g exact-ish. Epilogue:
    sqrt(max(feat + 2*CENTER - 2*hmax, eps)), Newton-refined ACT sqrt.
  - device (mode="pe_bias"): instead accumulates -csq/2 into PSUM via a
    K=1 DoubleRow pair (64*q0 + 8*q1 fp8 decomposition) at +25% PE cost;
    DVE does one full reduce from PSUM. Kept for A/B testing.
  - host: gather per-core [128, NT] outputs, reshape (no unpermute needed
    - only distance values are returned, invariant to centroid order).

Engine budget per core (98 tiles, HW-measured primitives): PE 16 DoubleRow
matmuls/tile -> ~321us; DVE (1920+128+128)c/tile + block finalize
-> ~273us; ACT/DMA hidden (et is fully SBUF-resident); the 3-op block
finalize is software-pipelined into the next block's tiles so it never
stalls the PSUM-releasing reduces. Measured ~400us/iter (three window
medians 379/397/434) on the 8-core SPMD run by the unbiased two-K slope
method (the bf16 baseline re-measures ~1.11ms by the same method). Max rel err 3.0e-3 vs the fp32
reference (tolerance 2e-2): fp8 cross noise ~2.3e-3, group-bias ~1e-3.
"""

import numpy as np
import ml_dtypes

import concourse.bass as bass
import concourse.mybir as mybir
import concourse.tile as tile
from concourse import bacc
from concourse.bass_utils import run_bass_kernel_spmd

# Problem constants (hardcoded per harness contract)
B, N, D, M = 32, 3136, 1024, 2048
N_CORES = 8
B_PER_CORE = B // N_CORES            # 4
R = B_PER_CORE * N                   # 12544 rows per core
NT = R // 128                        # 98 row tiles per core
KC = D // 128                        # 8 contraction chunks
KP = KC // 2                         # 4 DoubleRow k-pair chunks
NC_CHUNKS = M // 512                 # 4 PSUM chunks of 512 centroids
FP_H = 56
XW = 128                             # extreme-csq centroids handled exactly
GSZ = 8                              # centroids per group (gsort mode)
MMID = M - XW                        # grouped centroids
G = MMID // GSZ                      # 240 groups

BF16 = mybir.dt.bfloat16
FP8 = mybir.dt.float8e4
F32 = mybir.dt.float32
NP_BF16 = ml_dtypes.bfloat16
NP_FP8 = ml_dtypes.float8_e4m3

BIAS_S0 = 64.0   # coarse bias term scale (pe_bias mode)
BIAS_S1 = 8.0    # residual bias term scale (pe_bias mode)
CENTER = 512.0   # group-bias centering (gsort mode, keeps brep bf16 exact-ish)

MODE = "gsort"   # "gsort" | "gsact" | "pe_bias"


def build_program(n_row_tiles=NT, block_tiles=7, n_iters=1, n_devices=N_CORES,
                  enable_asserts=False, mode=None):
    """Build + compile the SPMD bass program.

    n_row_tiles: row tiles (128 rows each) processed per core.
    block_tiles: row tiles per DMA block (must divide n_row_tiles).
    n_iters: repeat whole compute (for loop-delta timing), python-unrolled.
    """
    mode = mode or MODE
    assert n_row_tiles % block_tiles == 0
    n_blocks = n_row_tiles // block_tiles
    rows = n_row_tiles * 128
    blk = block_tiles * 128

    nc = bacc.Bacc("TRN2", target_bir_lowering=False, debug=False,
                   num_devices=n_devices, enable_asserts=enable_asserts)

    et = nc.dram_tensor("et", [D, rows], FP8, kind="ExternalInput").ap()
    ct = nc.dram_tensor("ct", [D, M], FP8, kind="ExternalInput").ap()
    bias = nc.dram_tensor("bias", [1, 2, M], FP8, kind="ExternalInput").ap()
    bones = nc.dram_tensor("bones", [1, 2, 128], FP8, kind="ExternalInput").ap()
    brep = nc.dram_tensor("brep", [128, block_tiles, G], BF16,
                          kind="ExternalInput").ap()
    csqx = nc.dram_tensor("csqx", [128, XW], F32, kind="ExternalInput").ap()
    biasx = nc.dram_tensor("biasx", [1, 2, 512], FP8,
                           kind="ExternalInput").ap()
    feat = nc.dram_tensor("feat", [128, n_row_tiles], F32,
                          kind="ExternalInput").ap()
    out = nc.dram_tensor("out", [128, n_row_tiles], F32,
                         kind="ExternalOutput").ap()

    with tile.TileContext(nc) as tc:
        psum_bufs = 4 if mode == "gsort" else 2
        with (
            tc.tile_pool(name="const", bufs=1) as const_pool,
            tc.tile_pool(name="etp", bufs=2) as et_pool,
            tc.tile_pool(name="psum", bufs=psum_bufs, space="PSUM") as psum_pool,
            tc.tile_pool(name="gmp", bufs=2) as gm_pool,
            tc.tile_pool(name="hm", bufs=2) as hm_pool,
            tc.tile_pool(name="epi", bufs=1) as epi_pool,
        ):
            ct_sb = const_pool.tile([128, KC, M], FP8)
            bias_sb = const_pool.tile([1, 2, M], FP8)
            bones_sb = const_pool.tile([1, 2, 128], FP8)
            brep_sb = const_pool.tile([128, block_tiles, G], BF16)
            csqx_sb = const_pool.tile([128, XW], F32)
            feat_sb = const_pool.tile([128, n_row_tiles], F32)
            hmax_sb = const_pool.tile([128, n_row_tiles], F32)
            # et fully resident (12.8 MB < SBUF): block-granular DMAs so the
            # first tiles only wait for their own block, not the whole load
            eta_sb = const_pool.tile([128, KC, rows], FP8)
            for b in range(n_blocks):
                for k in range(KC):
                    nc.sync.dma_start(
                        eta_sb[:, k, b * blk:(b + 1) * blk],
                        et[k * 128:(k + 1) * 128, b * blk:(b + 1) * blk])
            for k in range(KC):
                nc.sync.dma_start(ct_sb[:, k, :], ct[k * 128:(k + 1) * 128, :])
            if mode == "pe_bias":
                nc.sync.dma_start(bias_sb[:], bias[:, :, :])
                nc.sync.dma_start(bones_sb[:], bones[:, :, :])
            else:
                nc.sync.dma_start(brep_sb[:], brep[:, :, :])
                nc.sync.dma_start(csqx_sb[:], csqx[:, :])
            nc.sync.dma_start(feat_sb[:], feat[:, :])

            def _fin_ops(b, gm, hx):
                """3-step finalize for block b: yields one DVE op per call,
                interleaved between the NEXT block's tile reduces so the
                burst never queues ahead of PSUM-releasing ops."""
                s, w = b * block_tiles, block_tiles
                hm = hm_pool.tile([128, w, G], F32)
                nc.vector.tensor_sub(hm[:], gm[:], brep_sb[:])
                yield
                hg = hm_pool.tile([128, w], F32)
                nc.vector.tensor_reduce(
                    hg[:], hm[:],
                    mybir.AxisListType.X, mybir.AluOpType.max)
                yield
                nc.vector.tensor_tensor(
                    hmax_sb[:, s:s + w], hg[:], hx[:],
                    mybir.AluOpType.max)

            def body(_it=None):
                pend = [None]

                def _step():
                    if pend[0] is not None:
                        try:
                            next(pend[0])
                        except StopIteration:
                            pend[0] = None

                def _flush():
                    while pend[0] is not None:
                        _step()

                for b in range(n_blocks):
                    if mode != "gsort":
                        et_sb = et_pool.tile([128, KC, blk], FP8)
                        for k in range(KC):
                            nc.sync.dma_start(
                                et_sb[:, k, :],
                                et[k * 128:(k + 1) * 128,
                                   b * blk:(b + 1) * blk])
                    gm = gm_pool.tile([128, block_tiles, G], BF16)
                    hx = gm_pool.tile([128, block_tiles], F32)
                    for j in range(block_tiles):
                        t = b * block_tiles + j
                        if mode == "gsort":
                            # half-width PSUM tiles (4 bufs): PE fills one
                            # half while DVE drains the other — this is what
                            # lets the DVE PSUM pass hide under the PE
                            lhsTs = [eta_sb[:, 2 * kp:2 * kp + 2,
                                            t * 128:(t + 1) * 128]
                                     for kp in range(KP)]
                            for h in range(2):
                                ph = psum_pool.tile([128, M // 2], F32)
                                for kp in range(KP):
                                    for n in range(2):
                                        nc.tensor.matmul(
                                            ph[:, n * 512:(n + 1) * 512],
                                            lhsTs[kp],
                                            ct_sb[:, 2 * kp:2 * kp + 2,
                                                  (2 * h + n) * 512:
                                                  (2 * h + n + 1) * 512],
                                            start=(kp == 0),
                                            stop=(kp == KP - 1),
                                            perf_mode=(
                                                mybir.MatmulPerfMode.DoubleRow))
                                if h == 0:
                                    nc.vector.tensor_reduce(
                                        gm[:, j, 0:128],
                                        ph[:].rearrange(
                                            "p (g i) -> p g i", g=128),
                                        mybir.AxisListType.X,
                                        mybir.AluOpType.max)
                                else:
                                    nc.vector.tensor_reduce(
                                        gm[:, j, 128:G],
                                        ph[:, 0:MMID - 1024].rearrange(
                                            "p (g i) -> p g i", g=G - 128),
                                        mybir.AxisListType.X,
                                        mybir.AluOpType.max)
                                    hxb = hm_pool.tile([128, XW], F32)
                                    nc.vector.tensor_sub(
                                        hxb[:], ph[:, MMID - 1024:],
                                        csqx_sb[:])
                                    nc.vector.tensor_reduce(
                                        hx[:, j:j + 1], hxb[:],
                                        mybir.AxisListType.X,
                                        mybir.AluOpType.max)
                            _step()
                            continue
                        ps = psum_pool.tile([128, M], F32)
                        if mode == "pe_bias":
                            # PSUM := -csq/2 (one K=1 DoubleRow pair/chunk)
                            for n in range(NC_CHUNKS):
                                nc.tensor.matmul(
                                    ps[:, n * 512:(n + 1) * 512],
                                    bones_sb[:, :, :],
                                    bias_sb[:, :, n * 512:(n + 1) * 512],
                                    start=True, stop=False,
                                    perf_mode=mybir.MatmulPerfMode.DoubleRow)
                        # PSUM += e.c (4 DoubleRow k-pair chunks)
                        for kp in range(KP):
                            lhsT = et_sb[:, 2 * kp:2 * kp + 2,
                                         j * 128:(j + 1) * 128]
                            for n in range(NC_CHUNKS):
                                nc.tensor.matmul(
                                    ps[:, n * 512:(n + 1) * 512],
                                    lhsT,
                                    ct_sb[:, 2 * kp:2 * kp + 2,
                                          n * 512:(n + 1) * 512],
                                    start=(mode != "pe_bias" and kp == 0),
                                    stop=(kp == KP - 1),
                                    perf_mode=mybir.MatmulPerfMode.DoubleRow)
                        if mode == "pe_bias":
                            nc.vector.tensor_reduce(
                                hmax_sb[:, t:t + 1], ps[:],
                                mybir.AxisListType.X, mybir.AluOpType.max)
                        else:
                            src = ps
                            if mode == "gsact":
                                # ACT evacuates PSUM; DVE works from SBUF
                                hb = hm_pool.tile([128, M], BF16)
                                nc.scalar.copy(hb[:, 0:1024], ps[:, 0:1024])
                                nc.scalar.copy(hb[:, 1024:M], ps[:, 1024:M])
                                src = hb
                            # per-group maxes of raw cross, single pass
                            nc.vector.tensor_reduce(
                                gm[:, j, :],
                                src[:, 0:MMID].rearrange(
                                    "p (g i) -> p g i", g=G),
                                mybir.AxisListType.X, mybir.AluOpType.max)
                            # exact pass over the extreme-csq tail columns
                            hxb = hm_pool.tile([128, XW], F32)
                            nc.vector.tensor_sub(
                                hxb[:], src[:, MMID:M], csqx_sb[:])
                            nc.vector.tensor_reduce(
                                hx[:, j:j + 1], hxb[:],
                                mybir.AxisListType.X, mybir.AluOpType.max)
                    if mode == "gsort":
                        _flush()
                        pend[:] = [_fin_ops(b, gm, hx)]
                    elif mode == "gsact":
                        # finalize block: hmax = max(max_g(gm - b_g), hx)
                        s, w = b * block_tiles, block_tiles
                        hm = hm_pool.tile([128, w, G], F32)
                        nc.vector.tensor_sub(hm[:], gm[:], brep_sb[:])
                        hg = hm_pool.tile([128, w], F32)
                        nc.vector.tensor_reduce(
                            hg[:], hm[:],
                            mybir.AxisListType.X, mybir.AluOpType.max)
                        nc.vector.tensor_tensor(
                            hmax_sb[:, s:s + w], hg[:], hx[:],
                            mybir.AluOpType.max)

                _flush()

                # epilogue: dist = sqrt(max(feat - 2*hmax, eps)), Newton-refined
                d2 = epi_pool.tile([128, n_row_tiles], F32)
                nc.vector.scalar_tensor_tensor(
                    out=d2[:], in0=hmax_sb[:], scalar=-2.0, in1=feat_sb[:],
                    op0=mybir.AluOpType.mult, op1=mybir.AluOpType.add)
                d2c = epi_pool.tile([128, n_row_tiles], F32)
                nc.vector.tensor_scalar_max(d2c[:], d2[:], 1.0e-12)
                s0 = epi_pool.tile([128, n_row_tiles], F32)
                nc.scalar.activation(s0[:], d2c[:],
                                     mybir.ActivationFunctionType.Sqrt)
                rcp = epi_pool.tile([128, n_row_tiles], F32)
                nc.vector.reciprocal(rcp[:], s0[:])
                q = epi_pool.tile([128, n_row_tiles], F32)
                nc.vector.tensor_mul(q[:], d2c[:], rcp[:])
                sq = epi_pool.tile([128, n_row_tiles], F32)
                nc.vector.tensor_add(sq[:], s0[:], q[:])
                res = epi_pool.tile([128, n_row_tiles], F32)
                nc.vector.tensor_scalar_mul(res[:], sq[:], 0.5)
                nc.sync.dma_start(out[:, :], res[:])

            # python-unrolled repetitions (For_i's back-edge machinery has
            # crashed the exec unit on this terminal; unrolled is safe)
            for _ in range(n_iters):
                body()

    nc.compile()
    return nc


_NC_CACHE = {}


def _get_program(key=(NT, 7, 1, N_CORES)):
    if key not in _NC_CACHE:
        _NC_CACHE[key] = build_program(*key)
    return _NC_CACHE[key]


def _bias_rows(csq):
    """Two-term fp8 decomposition of -||c||^2/2: BIAS_S0*q0 + BIAS_S1*q1."""
    x = (-0.5 * csq).astype(np.float32)
    q0 = (x / BIAS_S0).astype(NP_FP8)
    r = x - BIAS_S0 * q0.astype(np.float32)
    q1 = (r / BIAS_S1).astype(NP_FP8)
    return np.stack([q0, q1])[None]                                # [1, 2, M]


def prep_const(centroids, block_tiles=7):
    """Centroid-side tensors: sort by csq (extremes last), fp8 transpose,
    per-group midrange biases + exact biases for the extreme tail."""
    c64 = np.asarray(centroids).astype(np.float64)
    csq0 = np.einsum("md,md->m", c64, c64)
    order = np.argsort(csq0, kind="stable")
    order = np.concatenate(
        [order[XW // 2:-XW // 2], order[:XW // 2], order[-XW // 2:]])
    c64 = c64[order]
    csq = csq0[order]
    ct_np = np.ascontiguousarray(c64.astype(np.float32).astype(NP_FP8).T)
    bias_np = _bias_rows(csq)
    bones_np = np.empty((1, 2, 128), NP_FP8)
    bones_np[0, 0, :] = NP_FP8(BIAS_S0)
    bones_np[0, 1, :] = NP_FP8(BIAS_S1)
    half = 0.5 * csq[:MMID].reshape(MMID // GSZ, GSZ)
    b_g = 0.5 * (half.min(1) + half.max(1)) - CENTER               # midrange
    brep_np = np.ascontiguousarray(np.broadcast_to(
        b_g.astype(np.float32)[None, None, :],
        (128, block_tiles, G))).astype(NP_BF16)
    csqx_np = np.ascontiguousarray(np.broadcast_to(
        (0.5 * csq[MMID:] - CENTER).astype(np.float32)[None, :], (128, XW)))
    biasx_np = np.zeros((1, 2, 512), NP_FP8)
    biasx_np[:, :, 512 - XW:] = _bias_rows(csq[MMID:])
    return {"ct": ct_np, "bias": bias_np, "bones": bones_np,
            "brep": brep_np, "csqx": csqx_np, "biasx": biasx_np}


def prep_rows(e_rows, n_row_tiles=NT):
    """Embedding-side tensors for one core: fp8 transpose + ||e||^2."""
    e = np.asarray(e_rows)
    et_np = np.ascontiguousarray(e.astype(NP_FP8).T)               # [D, R]
    f = np.einsum("rd,rd->r", e.astype(np.float64),
                  e.astype(np.float64)).astype(np.float32)
    if MODE in ("gsort", "gsact"):
        f = f + 2.0 * CENTER     # hmax comes back CENTER-shifted
    feat_np = np.ascontiguousarray(f.reshape(n_row_tiles, 128).T)
    return {"et": et_np, "feat": feat_np}


def prep_inputs(embeds, centroids):
    """Host-side shard + layout prep. Returns per-core input maps."""
    embeds = np.asarray(embeds)
    const = prep_const(centroids)
    in_maps = []
    for c in range(N_CORES):
        e = embeds[c * B_PER_CORE:(c + 1) * B_PER_CORE].reshape(R, D)
        m = dict(const)
        m.update(prep_rows(e))
        in_maps.append(m)
    return in_maps


def gather_output(results):
    """results: list of 8 dicts with 'out' [128, NT] -> [B, 1, 56, 56]."""
    per_core = [np.asarray(r["out"]).T.reshape(R) for r in results]
    sim = np.concatenate(per_core).reshape(B, N)
    return sim.reshape(B, FP_H, FP_H)[:, None, :, :].astype(np.float32)


def kernel(embeds, centroids):
    nc = _get_program()
    in_maps = prep_inputs(embeds, centroids)
    res = run_bass_kernel_spmd(nc, in_maps, list(range(N_CORES)))
    return gather_output(res.results)


class CachedRunner:
    """Low-overhead repeat runner: jit once, keep inputs resident on device.

    Mirrors bass2jax.run_bass_via_pjrt's multi-core path but caches the
    jitted callable and the device-side input shards so repeated calls pay
    only dispatch + execution (for timing measurements).
    """

    def __init__(self, nc, in_maps):
        import jax
        import concourse.mybir as _mybir
        from jax.sharding import Mesh, PartitionSpec, NamedSharding
        from jax.experimental.shard_map import shard_map
        from concourse import bass2jax

        bass2jax.install_neuronx_cc_hook()
        n_cores = len(in_maps)
        partition_name = (nc.partition_id_tensor.name
                          if nc.partition_id_tensor else None)
        in_names, out_names, out_avals = [], [], []
        for alloc in nc.m.functions[0].allocations:
            if not isinstance(alloc, _mybir.MemoryLocationSet):
                continue
            name = alloc.memorylocations[0].name
            if alloc.kind == "ExternalInput":
                if name != partition_name:
                    in_names.append(name)
            elif alloc.kind == "ExternalOutput":
                shape = tuple(alloc.tensor_shape)
                dtype = _mybir.dt.np(alloc.dtype)
                out_names.append(name)
                out_avals.append(jax.core.ShapedArray(shape, dtype))
        n_params = len(in_names)
        all_in = in_names + out_names
        if partition_name is not None:
            all_in.append(partition_name)

        def _body(*args):
            operands = list(args)
            if partition_name is not None:
                operands.append(bass2jax.partition_id_tensor())
            outs = bass2jax._bass_exec_p.bind(
                *operands,
                out_avals=tuple(out_avals),
                in_names=tuple(all_in),
                out_names=tuple(out_names),
                lowering_input_output_aliases=(),
                sim_require_finite=True,
                sim_require_nnan=True,
                nc=nc,
            )
            return tuple(outs)

        devices = jax.devices()[:n_cores]
        mesh = Mesh(np.asarray(devices), ("core",))
        n_outs = len(out_names)
        donate = tuple(range(n_params, n_params + n_outs))
        self._fn = jax.jit(
            shard_map(_body, mesh=mesh,
                      in_specs=(PartitionSpec("core"),) * (n_params + n_outs),
                      out_specs=(PartitionSpec("core"),) * n_outs,
                      check_rep=False),
            donate_argnums=donate, keep_unused=True)
        sh = NamedSharding(mesh, PartitionSpec("core"))
        self._dev_in = [
            jax.device_put(
                np.concatenate([np.asarray(in_maps[c][nm])
                                for c in range(n_cores)], axis=0), sh)
            for nm in in_names]
        self._zero_shapes = [(n_cores * a.shape[0], *a.shape[1:])
                             for a in out_avals]
        self._zero_dtypes = [a.dtype for a in out_avals]
        self._out_names = out_names
        self._out_avals = out_avals
        self._n_cores = n_cores
        self._jax = jax

    def __call__(self):
        zeros = [np.zeros(s, d) for s, d in
                 zip(self._zero_shapes, self._zero_dtypes)]
        out = self._fn(*self._dev_in, *zeros)
        self._jax.block_until_ready(out)
        return out

    def results(self):
        out = self()
        return [
            {nm: np.asarray(out[i]).reshape(
                self._n_cores, *self._out_avals[i].shape)[c]
             for i, nm in enumerate(self._out_names)}
            for c in range(self._n_cores)]
